# revision 37
# baseline (speedup 1.0000x reference)
"""GAT layer on 8 trn2 NeuronCores (Bass/Tile).

Sharding: edges sorted by target node; each core owns a contiguous range of
V/8 target nodes and every edge pointing into it, so attention normalizers
and message sums are core-local (no all-reduce). Node features are projected
into per-core Q/V tables (full, replicated compute) and a core-local K table.

Per core:
  phase 1: TensorE computes Q|V rows (bf16, interleaved 512B records) for all
           nodes into a DRAM table, plus K rows for the core's own range.
  phase 2: per 128-target-node window: dma_gather QV[src] (table split at row
           32768 for the int16 gather index; window edges are grouped into
           low/high chunks of 128) and K[tgt] (rebased core-local indices);
           VectorE: per-edge logit = sum(Q[src]*K[tgt]) per head, bias =
           leaky_relu(ew*We+be), attention exp, message scaling; one-hot
           matrices (is_equal against an iota row) drive TensorE scatter
           matmuls accumulating [message | attn_exp] into PSUM per window;
           then normalize by degree, W_o matmul, leaky_relu, DMA out.

Host does only data movement: sorting/sharding/padding, dtype casts,
index wrapping, output concatenation.
"""

import sys, types, math
import numpy as np

try:
    import antenv.axon_hooks  # noqa: F401
except Exception:
    import antenv  # noqa: F401
    _ah = types.ModuleType("antenv.axon_hooks")
    _ah.get_axon_ntff_profile_hook = lambda: None
    sys.modules["antenv.axon_hooks"] = _ah

import concourse.bass as bass
import concourse.mybir as mybir
import concourse.tile as tile
from concourse import bacc
from concourse.masks import make_identity

P = 128
NCORES = 8
SPLIT = 32768
NEG_SLOPE = 0.2
BF = mybir.dt.bfloat16
F32 = mybir.dt.float32
I16 = mybir.dt.int16
AX = mybir.AxisListType
AF = mybir.ActivationFunctionType
OP = mybir.AluOpType
ABLATE = set()  # {'gather','scatter','dve','phase1'} for timing experiments


def _wrap_idx(pos_idx):
    """dma_gather idx layout: position i -> (partition i%16, col i//16),
    replicated across the 8 Q7 cores (128 partitions)."""
    n = len(pos_idx)
    n16 = (n + 15) // 16
    flat = np.zeros(n16 * 16, dtype=np.int16)
    flat[:n] = pos_idx
    w = flat.reshape(n16, 16).T.copy()
    return np.tile(w, (8, 1))


def _prep(h, edge_index, edge_weight):
    V, D = h.shape
    src = np.asarray(edge_index[0], dtype=np.int64)
    tgt = np.asarray(edge_index[1], dtype=np.int64)
    ew = np.asarray(edge_weight, dtype=np.float32)

    VPC = (V + NCORES - 1) // NCORES
    NW = (VPC + P - 1) // P
    KROWS = ((VPC + P - 1) // P) * P
    VTPAD = ((V + 1023) // 1024) * 1024

    order = np.argsort(tgt, kind="stable")
    s_src, s_tgt, s_ew = src[order], tgt[order], ew[order]
    core_id = s_tgt // VPC
    win_id = (s_tgt % VPC) // P

    lists = [[[None, None] for _ in range(NW)] for _ in range(NCORES)]
    for c in range(NCORES):
        m_c = core_id == c
        cs, ct, cw_, cwin = s_src[m_c], s_tgt[m_c], s_ew[m_c], win_id[m_c]
        for w in range(NW):
            m_w = cwin == w
            ws, wt, we_ = cs[m_w], ct[m_w], cw_[m_w]
            lo = ws < SPLIT
            lists[c][w][0] = (ws[lo], wt[lo], we_[lo])
            lists[c][w][1] = (ws[~lo] - SPLIT, wt[~lo], we_[~lo])

    nch = np.zeros((NW, 2), dtype=np.int64)
    nexact = np.zeros((NW, 2), dtype=np.int64)
    for w in range(NW):
        for hlf in range(2):
            mx = max(len(lists[c][w][hlf][0]) for c in range(NCORES))
            nch[w, hlf] = max(1, (mx + P - 1) // P)
            # exact gather count (max over cores, 16-aligned for the idx wrap)
            nexact[w, hlf] = max(16, ((mx + 15) // 16) * 16)
    nch_tot = int(nch.sum())
    nchmax = int(nch.sum(axis=1).max())
    # process big windows first: short dependency chains drain the pipeline
    worder = np.argsort(-(nch[:, 0] + nch[:, 1]), kind="stable")

    cores = []
    for c in range(NCORES):
        src16_cols, k16_cols = [], []
        tgtrel = np.full((P, nch_tot), -1e9, dtype=np.float32)
        ew4 = np.zeros((P, nch_tot), dtype=np.float32)
        ccol = 0
        for w in worder:
            kidx_all = []
            for hlf in range(2):
                ws, wt, we_ = lists[c][w][hlf]
                n_slots = int(nch[w, hlf]) * P
                sl_src = np.zeros(n_slots, dtype=np.int64)
                sl_src[: len(ws)] = ws
                sl_rel = np.full(n_slots, -1e9, dtype=np.float32)
                sl_rel[: len(wt)] = (wt - (c * VPC + w * P)).astype(np.float32)
                sl_ew = np.zeros(n_slots, dtype=np.float32)
                sl_ew[: len(we_)] = we_
                sl_k = np.zeros(n_slots, dtype=np.int64)
                sl_k[: len(wt)] = wt - c * VPC
                # match the (g p t) row order of the on-device Ktab writes
                kv = sl_k[: len(wt)]
                main = kv < (KROWS // 512) * 512
                kv_m = kv[main]
                kv[main] = (kv_m // 512) * 512 + (kv_m % 128) * 4 + (kv_m // 128) % 4
                sl_k[: len(wt)] = kv
                src16_cols.append(_wrap_idx(sl_src))
                kidx_all.append(sl_k)
                for j in range(int(nch[w, hlf])):
                    tgtrel[:, ccol + j] = sl_rel[j * P : (j + 1) * P]
                    ew4[:, ccol + j] = sl_ew[j * P : (j + 1) * P]
                ccol += int(nch[w, hlf])
            k16_cols.append(_wrap_idx(np.concatenate(kidx_all)))
        cores.append(
            dict(
                src16=np.ascontiguousarray(np.concatenate(src16_cols, axis=1)),
                k16=np.ascontiguousarray(np.concatenate(k16_cols, axis=1)),
                tgtrel=tgtrel,
                ew4=ew4,
            )
        )

    meta = dict(
        V=V, D=D, VPC=VPC, NW=NW, VTPAD=VTPAD, nch=nch, nch_tot=nch_tot,
        nchmax=nchmax, idx_cols=nch_tot * 8, nexact=nexact, worder=worder,
    )
    return cores, meta


def _build(meta, has_bqkv, has_bo, has_be=True):
    V, D = meta["V"], meta["D"]
    VPC, NW, VTPAD = meta["VPC"], meta["NW"], meta["VTPAD"]
    nch, nchmax, nch_tot = meta["nch"], meta["nchmax"], meta["nch_tot"]
    KROWS = NW * P
    INV_S = 1.0 / math.sqrt(D // 4)
    IDXC = meta["idx_cols"]

    nc = bacc.Bacc(None, target_bir_lowering=False)

    hT = nc.declare_dram_parameter("hT", [P, VTPAD], BF, isOutput=False)
    hKT = nc.declare_dram_parameter("hKT", [P, KROWS], BF, isOutput=False)
    Wqv = nc.declare_dram_parameter("Wqv", [P, 2 * D], BF, isOutput=False)
    Wk = nc.declare_dram_parameter("Wk", [P, D], BF, isOutput=False)
    Wo = nc.declare_dram_parameter("Wo", [P, D], BF, isOutput=False)
    We_t = nc.declare_dram_parameter("We_t", [P, nchmax * 4], F32, isOutput=False)
    be_t = nc.declare_dram_parameter("be_t", [P, nchmax * 4], F32, isOutput=False)
    bqkv_t = nc.declare_dram_parameter("bqkv_t", [P, 3 * D], F32, isOutput=False)
    bo_t = nc.declare_dram_parameter("bo_t", [P, D], F32, isOutput=False)
    iota_b = nc.declare_dram_parameter("iota_b", [P, P], BF, isOutput=False)
    src16 = nc.declare_dram_parameter("src16", [P, IDXC], I16, isOutput=False)
    k16 = nc.declare_dram_parameter("k16", [P, IDXC], I16, isOutput=False)
    tgtrel = nc.declare_dram_parameter("tgtrel", [P, nch_tot], F32, isOutput=False)
    ew4 = nc.declare_dram_parameter("ew4", [P, nch_tot], F32, isOutput=False)
    out = nc.declare_dram_parameter("out", [KROWS, D], F32, isOutput=True)

    QVtab = nc.dram_tensor("QVtab", [VTPAD, 2 * D], BF)
    Ktab = nc.dram_tensor("Ktab", [KROWS, D], BF)
    NT = VTPAD // P
    KT = KROWS // P

    with tile.TileContext(nc) as tc:
        with tc.tile_pool(name="const", bufs=1) as constp:
            wqv_t = constp.tile([P, 2 * D], BF)
            wk_t = constp.tile([P, D], BF)
            wo_t = constp.tile([P, D], BF)
            we_c = constp.tile([P, nchmax * 4], F32)
            be_c = constp.tile([P, nchmax * 4], F32)
            iota_c = constp.tile([P, P], BF)
            ident = constp.tile([P, P], BF)
            src16_c = constp.tile([P, IDXC], I16)
            k16_c = constp.tile([P, IDXC], I16)
            tgtrel_f = constp.tile([P, nch_tot], F32)
            ew4_c = constp.tile([P, nch_tot], F32)
            nc.sync.dma_start(out=wqv_t[:], in_=Wqv[:])
            make_identity(nc, ident[:])
            bqkv_c = constp.tile([P, 3 * D], F32)
            bo_c = constp.tile([P, D], F32)

            # ---------------- phase 1: projection tables ----------------
            with (
                tc.tile_pool(name="hbuf", bufs=1) as hbufp,
                tc.tile_pool(name="p1ps", bufs=3, space="PSUM") as p1ps,
                tc.tile_pool(name="p1psk", bufs=1, space="PSUM") as p1psk,
                tc.tile_pool(name="p1sb", bufs=10) as p1sb,
            ):
                hT_s = hbufp.tile([P, VTPAD], BF)
                NSLAB = 32
                slab = VTPAD // NSLAB
                for s in range(NSLAB):
                    nc.sync.dma_start(out=hT_s[:, s * slab : (s + 1) * slab],
                                      in_=hT[:, s * slab : (s + 1) * slab])
                hKT_s = hbufp.tile([P, KROWS], BF)
                nc.sync.dma_start(out=hKT_s[:], in_=hKT[:])
                # phase-2 constants: loaded after the h slabs so they never
                # delay the phase-1 critical path
                nc.sync.dma_start(out=wk_t[:], in_=Wk[:])
                nc.sync.dma_start(out=wo_t[:], in_=Wo[:])
                nc.sync.dma_start(out=we_c[:], in_=We_t[:])
                nc.sync.dma_start(out=be_c[:], in_=be_t[:])
                nc.sync.dma_start(out=iota_c[:], in_=iota_b[:])
                nc.sync.dma_start(out=src16_c[:], in_=src16[:])
                nc.sync.dma_start(out=k16_c[:], in_=k16[:])
                nc.sync.dma_start(out=tgtrel_f[:], in_=tgtrel[:])
                nc.sync.dma_start(out=ew4_c[:], in_=ew4[:])
                if has_bqkv:
                    nc.sync.dma_start(out=bqkv_c[:], in_=bqkv_t[:])
                if has_bo:
                    nc.sync.dma_start(out=bo_c[:], in_=bo_t[:])
                QVtab_r = QVtab.rearrange("(g t p) d -> g p t d", t=4, p=P)
                for g in range(NT // 4):
                    stage = p1sb.tile([P, 4, 2 * D], BF, tag="qvsb")
                    ps = p1ps.tile([P, 4, 2 * D], F32, tag="qvps")
                    for tt in range(4):
                        t = g * 4 + tt
                        lhs = hT_s[:, t * P : (t + 1) * P]
                        nc.tensor.matmul(out=ps[:, tt, :], lhsT=lhs, rhs=wqv_t[:],
                                         start=True, stop=True)
                    if has_bqkv:
                        for tt in range(4):
                            nc.vector.tensor_add(out=ps[:, tt, 0:D],
                                                 in0=ps[:, tt, 0:D],
                                                 in1=bqkv_c[:, 0:D])
                            nc.vector.tensor_add(out=ps[:, tt, D : 2 * D],
                                                 in0=ps[:, tt, D : 2 * D],
                                                 in1=bqkv_c[:, 2 * D : 3 * D])
                    if g % 2 == 0:
                        nc.scalar.copy(out=stage[:], in_=ps[:])
                    else:
                        nc.vector.tensor_copy(out=stage[:], in_=ps[:])
                    nc.sync.dma_start(out=QVtab_r[g], in_=stage[:])
                KG = KT // 4
                # (g p t) row order: each partition writes 4 consecutive
                # 256B rows = 1024B descriptors (no sub-512B DMA penalty)
                Ktab_r = (
                    Ktab[0 : KG * 4 * P, :].rearrange("(g p t) d -> g p t d", t=4, p=P)
                    if KG > 0 else None
                )
                for g in range(KG):
                    stage = p1sb.tile([P, 4, D], BF, tag="ksb")
                    ps = p1psk.tile([P, 4, D], F32, tag="kps")
                    for tt in range(4):
                        t = g * 4 + tt
                        lhs = hKT_s[:, t * P : (t + 1) * P]
                        nc.tensor.matmul(out=ps[:, tt, :], lhsT=lhs, rhs=wk_t[:],
                                         start=True, stop=True)
                    if has_bqkv:
                        for tt in range(4):
                            nc.vector.tensor_add(out=ps[:, tt, :], in0=ps[:, tt, :],
                                                 in1=bqkv_c[:, D : 2 * D])
                    if g % 2 == 0:
                        nc.scalar.copy(out=stage[:], in_=ps[:])
                    else:
                        nc.vector.tensor_copy(out=stage[:], in_=ps[:])
                    nc.sync.dma_start(out=Ktab_r[g], in_=stage[:])
                for t in range(KG * 4, KT):
                    ps = p1psk.tile([P, D], F32, tag="kps")
                    lhs = hKT_s[:, t * P : (t + 1) * P]
                    nc.tensor.matmul(out=ps[:], lhsT=lhs, rhs=wk_t[:],
                                     start=True, stop=True)
                    if has_bqkv:
                        nc.vector.tensor_add(out=ps[:], in0=ps[:],
                                             in1=bqkv_c[:, D : 2 * D])
                    kv = p1sb.tile([P, D], BF, tag="ksb1")
                    nc.vector.tensor_copy(out=kv[:], in_=ps[:])
                    nc.sync.dma_start(out=Ktab[t * P : (t + 1) * P, :], in_=kv[:])

            # ---------------- phase 2: edge processing ----------------
            nexact = meta["nexact"]
            with (
                tc.tile_pool(name="gth", bufs=5) as gthp,
                tc.tile_pool(name="work", bufs=3) as workp,
                tc.tile_pool(name="small", bufs=6) as smallp,
                tc.tile_pool(name="oh", bufs=8) as ohp,
                tc.tile_pool(name="accps", bufs=3, space="PSUM") as accps,
                tc.tile_pool(name="tailps", bufs=2, space="PSUM") as tailps,
            ):
                # tables must land in DRAM before any gather reads them
                tc.strict_bb_all_engine_barrier()

                qv_lo = QVtab[0:SPLIT, :]
                qv_hi = QVtab[SPLIT:VTPAD, :]
                c0 = 0      # chunk column cursor
                i16 = 0     # idx column cursor (same for src16 / k16)
                for w in meta["worder"]:
                    nl, nh = int(nch[w, 0]), int(nch[w, 1])
                    nel, neh = int(nexact[w, 0]), int(nexact[w, 1])
                    ncw = nl + nh
                    qv_g = gthp.tile([P, ncw, 2 * D], BF, tag="qvg")
                    k_g = gthp.tile([P, ncw, D], BF, tag="kg")
                    nc.gpsimd.dma_gather(
                        out_ap=qv_g[:, 0:nl, :], in_ap=qv_lo,
                        idxs_ap=src16_c[:, i16 : i16 + nl * 8],
                        num_idxs=nl * P, num_idxs_reg=nl * P, elem_size=2 * D,
                        single_packet=False,
                    )
                    nc.gpsimd.dma_gather(
                        out_ap=qv_g[:, nl:ncw, :], in_ap=qv_hi,
                        idxs_ap=src16_c[:, i16 + nl * 8 : i16 + ncw * 8],
                        num_idxs=nh * P, num_idxs_reg=nh * P, elem_size=2 * D,
                        single_packet=False,
                    )
                    nc.gpsimd.dma_gather(
                        out_ap=k_g[:], in_ap=Ktab[:],
                        idxs_ap=k16_c[:, i16 : i16 + ncw * 8],
                        num_idxs=ncw * P, num_idxs_reg=ncw * P, elem_size=D,
                        single_packet=False,
                    )

                    qk = workp.tile([P, ncw, D], BF, tag="qk")
                    nc.vector.tensor_mul(out=qk[:], in0=qv_g[:, :, 0:D], in1=k_g[:])
                    qk4 = qk[:].rearrange("p c (h f) -> p c h f", f=D // 4)
                    t16 = workp.tile([P, ncw, 4, 16], BF, tag="t16")
                    nc.vector.tensor_add(out=t16[:], in0=qk4[:, :, :, 0:16],
                                         in1=qk4[:, :, :, 16:32])
                    t8 = smallp.tile([P, ncw, 4, 8], BF, tag="t8")
                    nc.vector.tensor_add(out=t8[:], in0=t16[:, :, :, 0:8],
                                         in1=t16[:, :, :, 8:16])
                    t4f = smallp.tile([P, ncw, 4, 4], BF, tag="t4f")
                    nc.vector.tensor_add(out=t4f[:], in0=t8[:, :, :, 0:4],
                                         in1=t8[:, :, :, 4:8])
                    t2f = smallp.tile([P, ncw, 4, 2], BF, tag="t2f")
                    nc.vector.tensor_add(out=t2f[:], in0=t4f[:, :, :, 0:2],
                                         in1=t4f[:, :, :, 2:4])
                    logits = smallp.tile([P, ncw, 4], BF, tag="logits")
                    nc.vector.tensor_add(
                        out=logits[:].rearrange("p c (h x) -> p c h x", x=1),
                        in0=t2f[:, :, :, 0:1], in1=t2f[:, :, :, 1:2])
                    ewс = ew4_c[:, c0 : c0 + ncw].to_broadcast([P, ncw, 4])
                    biasp = smallp.tile([P, ncw * 4], F32, tag="biasp")
                    nc.vector.tensor_mul(
                        out=biasp[:].rearrange("p (c h) -> p c h", h=4),
                        in0=ewс,
                        in1=we_c[:, 0 : ncw * 4].rearrange("p (c h) -> p c h", h=4))
                    if has_be:
                        nc.vector.tensor_add(out=biasp[:], in0=biasp[:],
                                             in1=be_c[:, 0 : ncw * 4])
                    ebx = smallp.tile([P, ncw * 4], F32, tag="ebx")
                    nc.vector.tensor_scalar_mul(out=ebx[:], in0=biasp[:],
                                                scalar1=NEG_SLOPE)
                    nc.vector.tensor_tensor(out=ebx[:], in0=biasp[:], in1=ebx[:],
                                            op=OP.max)
                    nc.scalar.activation(out=ebx[:], in_=ebx[:], func=AF.Exp)
                    el = smallp.tile([P, ncw * 4], F32, tag="el")
                    nc.scalar.activation(
                        out=el[:],
                        in_=logits[:].rearrange("p c h -> p (c h)"),
                        func=AF.Exp, scale=INV_S,
                    )
                    attn_e = smallp.tile([P, ncw * 4], F32, tag="attne")
                    nc.vector.tensor_mul(out=attn_e[:], in0=el[:], in1=ebx[:])
                    attn_w = smallp.tile([P, ncw * 4], F32, tag="attnw")
                    nc.vector.tensor_mul(
                        out=attn_w[:].rearrange("p (c h) -> p c h", h=4),
                        in0=attn_e[:].rearrange("p (c h) -> p c h", h=4), in1=ewс)
                    ae_b = smallp.tile([P, ncw * 4], BF, tag="aeb")
                    nc.vector.tensor_copy(out=ae_b[:], in_=attn_e[:])
                    aw_x = workp.tile([P, ncw, D], BF, tag="awx")
                    nc.scalar.copy(
                        out=aw_x[:].rearrange("p c (h f) -> p c h f", f=D // 4),
                        in_=attn_w[:]
                        .rearrange("p (c h) -> p c h", h=4)
                        .to_broadcast([P, ncw, 4, D // 4]),
                    )
                    rhs = workp.tile([P, ncw, D + 4], BF, tag="rhs")
                    nc.vector.tensor_mul(out=rhs[:, :, 0:D], in0=aw_x[:],
                                         in1=qv_g[:, :, D : 2 * D])
                    nc.vector.tensor_copy(
                        out=rhs[:, :, D : D + 4],
                        in_=ae_b[:].rearrange("p (c h) -> p c h", h=4),
                    )

                    acc = accps.tile([P, D + 4], F32, tag="acc")
                    for c in range(ncw):
                        oh = ohp.tile([P, P], BF, tag="oh")
                        nc.vector.tensor_scalar(
                            out=oh[:], in0=iota_c[:],
                            scalar1=tgtrel_f[:, c0 + c : c0 + c + 1], scalar2=None,
                            op0=OP.is_equal,
                        )
                        nc.tensor.matmul(out=acc[:], lhsT=oh[:], rhs=rhs[:, c, :],
                                         start=(c == 0), stop=(c == ncw - 1))

                    r4 = smallp.tile([P, 4], F32, tag="r4")
                    nc.vector.tensor_scalar_add(out=r4[:], in0=acc[:, D : D + 4],
                                                scalar1=1e-16)
                    nc.vector.reciprocal(out=r4[:], in_=r4[:])
                    mn = smallp.tile([P, D], BF, tag="mn")
                    nc.vector.tensor_tensor(
                        out=mn[:].rearrange("p (h f) -> p h f", h=4),
                        in0=acc[:, 0:D].rearrange("p (h f) -> p h f", h=4),
                        in1=r4[:].to_broadcast([P, 4, D // 4]),
                        op=OP.mult,
                    )
                    pst = tailps.tile([P, P], BF, tag="pst")
                    nc.tensor.transpose(out=pst[:], in_=mn[:], identity=ident[:])
                    mT = smallp.tile([P, P], BF, tag="mT")
                    nc.scalar.copy(out=mT[:], in_=pst[:])
                    pso = tailps.tile([P, D], F32, tag="pso")
                    nc.tensor.matmul(out=pso[:], lhsT=mT[:], rhs=wo_t[:],
                                     start=True, stop=True)
                    res = smallp.tile([P, D], F32, tag="res")
                    res2 = smallp.tile([P, D], F32, tag="res2")
                    if has_bo:
                        nc.vector.tensor_add(out=res[:], in0=pso[:], in1=bo_c[:])
                        nc.scalar.mul(out=res2[:], in_=res[:], mul=NEG_SLOPE)
                        nc.vector.tensor_tensor(out=res[:], in0=res[:], in1=res2[:],
                                                op=OP.max)
                    else:
                        nc.scalar.mul(out=res2[:], in_=pso[:], mul=NEG_SLOPE)
                        nc.vector.tensor_tensor(out=res[:], in0=pso[:], in1=res2[:],
                                                op=OP.max)
                    nc.sync.dma_start(out=out[w * P : (w + 1) * P, :], in_=res[:])

                    c0 += ncw
                    i16 += ncw * 8

    nc.compile()
    return nc


def kernel(h, edge_index, edge_weight, Wq, bq, Wk, bk, Wv, bv, Wo, bo, We, be,
           _run=None):
    from concourse.bass_utils import run_bass_kernel_spmd

    h = np.asarray(h, dtype=np.float32)
    Wq, Wk, Wv, Wo = (np.asarray(x, dtype=np.float32) for x in (Wq, Wk, Wv, Wo))
    bq, bk, bv, bo = (np.asarray(x, dtype=np.float32) for x in (bq, bk, bv, bo))
    We = np.asarray(We, dtype=np.float32).reshape(1, 4)
    be = np.asarray(be, dtype=np.float32)
    V, D = h.shape

    cores, meta = _prep(h, edge_index, edge_weight)
    VPC, NW, VTPAD = meta["VPC"], meta["NW"], meta["VTPAD"]
    KROWS = NW * P
    nchmax = meta["nchmax"]

    has_bqkv = any(np.any(b != 0) for b in (bq, bk, bv))
    has_bo = bool(np.any(bo != 0))
    nc = _build(meta, has_bqkv, has_bo, has_be=bool(np.any(be != 0)))

    hT_np = np.zeros((P, VTPAD), dtype=np.float32)
    hT_np[:, :V] = h.T
    hT_np = hT_np.astype(mybir.dt.np(BF))
    iota_np = np.tile(np.arange(P, dtype=np.float32)[None, :], (P, 1)).astype(
        mybir.dt.np(BF)
    )
    we_np = np.tile(We.reshape(1, 4), (P, nchmax)).astype(np.float32)
    be_np = np.tile(be.reshape(1, 4), (P, nchmax)).astype(np.float32)
    bqkv_np = np.tile(
        np.concatenate([bq, bk, bv]).reshape(1, 3 * D), (P, 1)
    ).astype(np.float32)
    bo_np = np.tile(bo.reshape(1, D), (P, 1)).astype(np.float32)
    common = dict(
        hT=hT_np,
        Wqv=np.concatenate([Wq, Wv], axis=1).astype(mybir.dt.np(BF)),
        Wk=Wk.astype(mybir.dt.np(BF)),
        Wo=Wo.astype(mybir.dt.np(BF)),
        We_t=we_np, be_t=be_np, bqkv_t=bqkv_np, bo_t=bo_np, iota_b=iota_np,
    )
    in_maps = []
    for c in range(NCORES):
        hk = np.zeros((P, KROWS), dtype=np.float32)
        n0, n1 = c * VPC, min((c + 1) * VPC, V)
        hk[:, : n1 - n0] = h[n0:n1].T
        m = dict(common)
        m.update(
            hKT=hk.astype(mybir.dt.np(BF)),
            src16=cores[c]["src16"],
            k16=cores[c]["k16"],
            tgtrel=cores[c]["tgtrel"],
            ew4=cores[c]["ew4"],
        )
        in_maps.append(m)

    if _run is not None:          # test hook (CoreSim etc.)
        return _run(nc, in_maps, meta)

    # the axon transport occasionally fails transiently; one retry is cheap
    try:
        res = run_bass_kernel_spmd(nc, in_maps, core_ids=list(range(NCORES)))
    except Exception:
        res = run_bass_kernel_spmd(nc, in_maps, core_ids=list(range(NCORES)))
    outs = [r["out"][:VPC] for r in res.results]
    return np.concatenate(outs, axis=0)[:V].astype(np.float32)



# revision 38
# speedup vs baseline: 1.0075x; 1.0075x over previous
"""GAT layer on 8 trn2 NeuronCores (Bass/Tile).

Sharding: edges sorted by target node; each core owns a contiguous range of
V/8 target nodes and every edge pointing into it, so attention normalizers
and message sums are core-local (no all-reduce). Node features are projected
into per-core Q/V tables (full, replicated compute) and a core-local K table.

Per core:
  phase 1: TensorE computes Q|V rows (bf16, interleaved 512B records) for all
           nodes into a DRAM table, plus K rows for the core's own range.
  phase 2: per 128-target-node window: dma_gather QV[src] (table split at row
           32768 for the int16 gather index; window edges are grouped into
           low/high chunks of 128) and K[tgt] (rebased core-local indices);
           VectorE: per-edge logit = sum(Q[src]*K[tgt]) per head, bias =
           leaky_relu(ew*We+be), attention exp, message scaling; one-hot
           matrices (is_equal against an iota row) drive TensorE scatter
           matmuls accumulating [message | attn_exp] into PSUM per window;
           then normalize by degree, W_o matmul, leaky_relu, DMA out.

Host does only data movement: sorting/sharding/padding, dtype casts,
index wrapping, output concatenation.
"""

import sys, types, math
import numpy as np

try:
    import antenv.axon_hooks  # noqa: F401
except Exception:
    import antenv  # noqa: F401
    _ah = types.ModuleType("antenv.axon_hooks")
    _ah.get_axon_ntff_profile_hook = lambda: None
    sys.modules["antenv.axon_hooks"] = _ah

import concourse.bass as bass
import concourse.mybir as mybir
import concourse.tile as tile
from concourse import bacc
from concourse.masks import make_identity

P = 128
NCORES = 8
SPLIT = 32768
NEG_SLOPE = 0.2
BF = mybir.dt.bfloat16
F32 = mybir.dt.float32
I16 = mybir.dt.int16
AX = mybir.AxisListType
AF = mybir.ActivationFunctionType
OP = mybir.AluOpType
ABLATE = set()  # {'gather','scatter','dve','phase1'} for timing experiments


def _wrap_idx(pos_idx):
    """dma_gather idx layout: position i -> (partition i%16, col i//16),
    replicated across the 8 Q7 cores (128 partitions)."""
    n = len(pos_idx)
    n16 = (n + 15) // 16
    flat = np.zeros(n16 * 16, dtype=np.int16)
    flat[:n] = pos_idx
    w = flat.reshape(n16, 16).T.copy()
    return np.tile(w, (8, 1))


def _prep(h, edge_index, edge_weight):
    V, D = h.shape
    src = np.asarray(edge_index[0], dtype=np.int64)
    tgt = np.asarray(edge_index[1], dtype=np.int64)
    ew = np.asarray(edge_weight, dtype=np.float32)

    VPC = (V + NCORES - 1) // NCORES
    NW = (VPC + P - 1) // P
    KROWS = ((VPC + P - 1) // P) * P
    VTPAD = ((V + 1023) // 1024) * 1024

    order = np.argsort(tgt, kind="stable")
    s_src, s_tgt, s_ew = src[order], tgt[order], ew[order]
    core_id = s_tgt // VPC
    win_id = (s_tgt % VPC) // P

    lists = [[[None, None] for _ in range(NW)] for _ in range(NCORES)]
    for c in range(NCORES):
        m_c = core_id == c
        cs, ct, cw_, cwin = s_src[m_c], s_tgt[m_c], s_ew[m_c], win_id[m_c]
        # per-core rotated table position: own node range sits at col 0, so
        # the kernel reads K-projection inputs from hT at fixed offsets
        cs = (cs - c * VPC) % V
        for w in range(NW):
            m_w = cwin == w
            ws, wt, we_ = cs[m_w], ct[m_w], cw_[m_w]
            lo = ws < SPLIT
            lists[c][w][0] = (ws[lo], wt[lo], we_[lo])
            lists[c][w][1] = (ws[~lo] - SPLIT, wt[~lo], we_[~lo])

    nch = np.zeros((NW, 2), dtype=np.int64)
    nexact = np.zeros((NW, 2), dtype=np.int64)
    for w in range(NW):
        for hlf in range(2):
            mx = max(len(lists[c][w][hlf][0]) for c in range(NCORES))
            nch[w, hlf] = max(1, (mx + P - 1) // P)
            # exact gather count (max over cores, 16-aligned for the idx wrap)
            nexact[w, hlf] = max(16, ((mx + 15) // 16) * 16)
    nch_tot = int(nch.sum())
    nchmax = int(nch.sum(axis=1).max())
    # process big windows first: short dependency chains drain the pipeline
    worder = np.argsort(-(nch[:, 0] + nch[:, 1]), kind="stable")

    cores = []
    for c in range(NCORES):
        src16_cols, k16_cols = [], []
        tgtrel = np.full((P, nch_tot), -1e9, dtype=np.float32)
        ew4 = np.zeros((P, nch_tot), dtype=np.float32)
        ccol = 0
        for w in worder:
            kidx_all = []
            for hlf in range(2):
                ws, wt, we_ = lists[c][w][hlf]
                n_slots = int(nch[w, hlf]) * P
                sl_src = np.zeros(n_slots, dtype=np.int64)
                sl_src[: len(ws)] = ws
                sl_rel = np.full(n_slots, -1e9, dtype=np.float32)
                sl_rel[: len(wt)] = (wt - (c * VPC + w * P)).astype(np.float32)
                sl_ew = np.zeros(n_slots, dtype=np.float32)
                sl_ew[: len(we_)] = we_
                sl_k = np.zeros(n_slots, dtype=np.int64)
                sl_k[: len(wt)] = wt - c * VPC
                # match the (g p t) row order of the on-device Ktab writes
                kv = sl_k[: len(wt)]
                main = kv < (KROWS // 512) * 512
                kv_m = kv[main]
                kv[main] = (kv_m // 512) * 512 + (kv_m % 128) * 4 + (kv_m // 128) % 4
                sl_k[: len(wt)] = kv
                src16_cols.append(_wrap_idx(sl_src))
                kidx_all.append(sl_k)
                for j in range(int(nch[w, hlf])):
                    tgtrel[:, ccol + j] = sl_rel[j * P : (j + 1) * P]
                    ew4[:, ccol + j] = sl_ew[j * P : (j + 1) * P]
                ccol += int(nch[w, hlf])
            k16_cols.append(_wrap_idx(np.concatenate(kidx_all)))
        cores.append(
            dict(
                src16=np.ascontiguousarray(np.concatenate(src16_cols, axis=1)),
                k16=np.ascontiguousarray(np.concatenate(k16_cols, axis=1)),
                tgtrel=tgtrel,
                ew4=ew4,
            )
        )

    meta = dict(
        V=V, D=D, VPC=VPC, NW=NW, VTPAD=VTPAD, nch=nch, nch_tot=nch_tot,
        nchmax=nchmax, idx_cols=nch_tot * 8, nexact=nexact, worder=worder,
    )
    return cores, meta


def _build(meta, has_bqkv, has_bo, has_be=True):
    V, D = meta["V"], meta["D"]
    VPC, NW, VTPAD = meta["VPC"], meta["NW"], meta["VTPAD"]
    nch, nchmax, nch_tot = meta["nch"], meta["nchmax"], meta["nch_tot"]
    KROWS = NW * P
    INV_S = 1.0 / math.sqrt(D // 4)
    IDXC = meta["idx_cols"]

    nc = bacc.Bacc(None, target_bir_lowering=False)

    hT = nc.declare_dram_parameter("hT", [P, VTPAD], BF, isOutput=False)
    Wqv = nc.declare_dram_parameter("Wqv", [P, 2 * D], BF, isOutput=False)
    Wk = nc.declare_dram_parameter("Wk", [P, D], BF, isOutput=False)
    Wo = nc.declare_dram_parameter("Wo", [P, D], BF, isOutput=False)
    We_t = nc.declare_dram_parameter("We_t", [P, nchmax * 4], F32, isOutput=False)
    be_t = nc.declare_dram_parameter("be_t", [P, nchmax * 4], F32, isOutput=False)
    bqkv_t = nc.declare_dram_parameter("bqkv_t", [P, 3 * D], F32, isOutput=False)
    bo_t = nc.declare_dram_parameter("bo_t", [P, D], F32, isOutput=False)
    iota_b = nc.declare_dram_parameter("iota_b", [P, P], BF, isOutput=False)
    src16 = nc.declare_dram_parameter("src16", [P, IDXC], I16, isOutput=False)
    k16 = nc.declare_dram_parameter("k16", [P, IDXC], I16, isOutput=False)
    tgtrel = nc.declare_dram_parameter("tgtrel", [P, nch_tot], F32, isOutput=False)
    ew4 = nc.declare_dram_parameter("ew4", [P, nch_tot], F32, isOutput=False)
    out = nc.declare_dram_parameter("out", [KROWS, D], F32, isOutput=True)

    QVtab = nc.dram_tensor("QVtab", [VTPAD, 2 * D], BF)
    Ktab = nc.dram_tensor("Ktab", [KROWS, D], BF)
    NT = VTPAD // P
    KT = KROWS // P

    with tile.TileContext(nc) as tc:
        with tc.tile_pool(name="const", bufs=1) as constp:
            wqv_t = constp.tile([P, 2 * D], BF)
            wk_t = constp.tile([P, D], BF)
            wo_t = constp.tile([P, D], BF)
            we_c = constp.tile([P, nchmax * 4], F32)
            be_c = constp.tile([P, nchmax * 4], F32)
            iota_c = constp.tile([P, P], BF)
            ident = constp.tile([P, P], BF)
            src16_c = constp.tile([P, IDXC], I16)
            k16_c = constp.tile([P, IDXC], I16)
            tgtrel_f = constp.tile([P, nch_tot], F32)
            ew4_c = constp.tile([P, nch_tot], F32)
            nc.sync.dma_start(out=wqv_t[:], in_=Wqv[:])
            make_identity(nc, ident[:])
            bqkv_c = constp.tile([P, 3 * D], F32)
            bo_c = constp.tile([P, D], F32)

            # ---------------- phase 1: projection tables ----------------
            with (
                tc.tile_pool(name="hbuf", bufs=1) as hbufp,
                tc.tile_pool(name="p1ps", bufs=3, space="PSUM") as p1ps,
                tc.tile_pool(name="p1psk", bufs=1, space="PSUM") as p1psk,
                tc.tile_pool(name="p1sb", bufs=10) as p1sb,
            ):
                hT_s = hbufp.tile([P, VTPAD], BF)
                NSLAB = 32
                slab = VTPAD // NSLAB
                for s in range(NSLAB):
                    nc.sync.dma_start(out=hT_s[:, s * slab : (s + 1) * slab],
                                      in_=hT[:, s * slab : (s + 1) * slab])
                # phase-2 constants: loaded after the h slabs so they never
                # delay the phase-1 critical path
                nc.sync.dma_start(out=wk_t[:], in_=Wk[:])
                nc.sync.dma_start(out=wo_t[:], in_=Wo[:])
                nc.sync.dma_start(out=we_c[:], in_=We_t[:])
                nc.sync.dma_start(out=be_c[:], in_=be_t[:])
                nc.sync.dma_start(out=iota_c[:], in_=iota_b[:])
                nc.sync.dma_start(out=src16_c[:], in_=src16[:])
                nc.sync.dma_start(out=k16_c[:], in_=k16[:])
                nc.sync.dma_start(out=tgtrel_f[:], in_=tgtrel[:])
                nc.sync.dma_start(out=ew4_c[:], in_=ew4[:])
                if has_bqkv:
                    nc.sync.dma_start(out=bqkv_c[:], in_=bqkv_t[:])
                if has_bo:
                    nc.sync.dma_start(out=bo_c[:], in_=bo_t[:])
                QVtab_r = QVtab.rearrange("(g t p) d -> g p t d", t=4, p=P)
                for g in range(NT // 4):
                    stage = p1sb.tile([P, 4, 2 * D], BF, tag="qvsb")
                    ps = p1ps.tile([P, 4, 2 * D], F32, tag="qvps")
                    for tt in range(4):
                        t = g * 4 + tt
                        lhs = hT_s[:, t * P : (t + 1) * P]
                        nc.tensor.matmul(out=ps[:, tt, :], lhsT=lhs, rhs=wqv_t[:],
                                         start=True, stop=True)
                    if has_bqkv:
                        for tt in range(4):
                            nc.vector.tensor_add(out=ps[:, tt, 0:D],
                                                 in0=ps[:, tt, 0:D],
                                                 in1=bqkv_c[:, 0:D])
                            nc.vector.tensor_add(out=ps[:, tt, D : 2 * D],
                                                 in0=ps[:, tt, D : 2 * D],
                                                 in1=bqkv_c[:, 2 * D : 3 * D])
                    if g % 2 == 0:
                        nc.scalar.copy(out=stage[:], in_=ps[:])
                    else:
                        nc.vector.tensor_copy(out=stage[:], in_=ps[:])
                    nc.sync.dma_start(out=QVtab_r[g], in_=stage[:])
                KG = KT // 4
                # (g p t) row order: each partition writes 4 consecutive
                # 256B rows = 1024B descriptors (no sub-512B DMA penalty)
                Ktab_r = (
                    Ktab[0 : KG * 4 * P, :].rearrange("(g p t) d -> g p t d", t=4, p=P)
                    if KG > 0 else None
                )
                for g in range(KG):
                    stage = p1sb.tile([P, 4, D], BF, tag="ksb")
                    ps = p1psk.tile([P, 4, D], F32, tag="kps")
                    for tt in range(4):
                        t = g * 4 + tt
                        lhs = hT_s[:, t * P : (t + 1) * P]
                        nc.tensor.matmul(out=ps[:, tt, :], lhsT=lhs, rhs=wk_t[:],
                                         start=True, stop=True)
                    if has_bqkv:
                        for tt in range(4):
                            nc.vector.tensor_add(out=ps[:, tt, :], in0=ps[:, tt, :],
                                                 in1=bqkv_c[:, D : 2 * D])
                    if g % 2 == 0:
                        nc.scalar.copy(out=stage[:], in_=ps[:])
                    else:
                        nc.vector.tensor_copy(out=stage[:], in_=ps[:])
                    nc.sync.dma_start(out=Ktab_r[g], in_=stage[:])
                for t in range(KG * 4, KT):
                    ps = p1psk.tile([P, D], F32, tag="kps")
                    lhs = hT_s[:, t * P : (t + 1) * P]
                    nc.tensor.matmul(out=ps[:], lhsT=lhs, rhs=wk_t[:],
                                     start=True, stop=True)
                    if has_bqkv:
                        nc.vector.tensor_add(out=ps[:], in0=ps[:],
                                             in1=bqkv_c[:, D : 2 * D])
                    kv = p1sb.tile([P, D], BF, tag="ksb1")
                    nc.vector.tensor_copy(out=kv[:], in_=ps[:])
                    nc.sync.dma_start(out=Ktab[t * P : (t + 1) * P, :], in_=kv[:])

            # ---------------- phase 2: edge processing ----------------
            nexact = meta["nexact"]
            with (
                tc.tile_pool(name="gth", bufs=5) as gthp,
                tc.tile_pool(name="work", bufs=3) as workp,
                tc.tile_pool(name="small", bufs=6) as smallp,
                tc.tile_pool(name="oh", bufs=8) as ohp,
                tc.tile_pool(name="accps", bufs=3, space="PSUM") as accps,
                tc.tile_pool(name="tailps", bufs=2, space="PSUM") as tailps,
            ):
                # tables must land in DRAM before any gather reads them
                tc.strict_bb_all_engine_barrier()

                qv_lo = QVtab[0:SPLIT, :]
                qv_hi = QVtab[SPLIT:VTPAD, :]
                c0 = 0      # chunk column cursor
                i16 = 0     # idx column cursor (same for src16 / k16)
                for w in meta["worder"]:
                    nl, nh = int(nch[w, 0]), int(nch[w, 1])
                    nel, neh = int(nexact[w, 0]), int(nexact[w, 1])
                    ncw = nl + nh
                    qv_g = gthp.tile([P, ncw, 2 * D], BF, tag="qvg")
                    k_g = gthp.tile([P, ncw, D], BF, tag="kg")
                    nc.gpsimd.dma_gather(
                        out_ap=qv_g[:, 0:nl, :], in_ap=qv_lo,
                        idxs_ap=src16_c[:, i16 : i16 + nl * 8],
                        num_idxs=nl * P, num_idxs_reg=nl * P, elem_size=2 * D,
                        single_packet=False,
                    )
                    nc.gpsimd.dma_gather(
                        out_ap=qv_g[:, nl:ncw, :], in_ap=qv_hi,
                        idxs_ap=src16_c[:, i16 + nl * 8 : i16 + ncw * 8],
                        num_idxs=nh * P, num_idxs_reg=nh * P, elem_size=2 * D,
                        single_packet=False,
                    )
                    nc.gpsimd.dma_gather(
                        out_ap=k_g[:], in_ap=Ktab[:],
                        idxs_ap=k16_c[:, i16 : i16 + ncw * 8],
                        num_idxs=ncw * P, num_idxs_reg=ncw * P, elem_size=D,
                        single_packet=False,
                    )

                    qk = workp.tile([P, ncw, D], BF, tag="qk")
                    nc.vector.tensor_mul(out=qk[:], in0=qv_g[:, :, 0:D], in1=k_g[:])
                    qk4 = qk[:].rearrange("p c (h f) -> p c h f", f=D // 4)
                    t16 = workp.tile([P, ncw, 4, 16], BF, tag="t16")
                    nc.vector.tensor_add(out=t16[:], in0=qk4[:, :, :, 0:16],
                                         in1=qk4[:, :, :, 16:32])
                    t8 = smallp.tile([P, ncw, 4, 8], BF, tag="t8")
                    nc.vector.tensor_add(out=t8[:], in0=t16[:, :, :, 0:8],
                                         in1=t16[:, :, :, 8:16])
                    t4f = smallp.tile([P, ncw, 4, 4], BF, tag="t4f")
                    nc.vector.tensor_add(out=t4f[:], in0=t8[:, :, :, 0:4],
                                         in1=t8[:, :, :, 4:8])
                    t2f = smallp.tile([P, ncw, 4, 2], BF, tag="t2f")
                    nc.vector.tensor_add(out=t2f[:], in0=t4f[:, :, :, 0:2],
                                         in1=t4f[:, :, :, 2:4])
                    logits = smallp.tile([P, ncw, 4], BF, tag="logits")
                    nc.vector.tensor_add(
                        out=logits[:].rearrange("p c (h x) -> p c h x", x=1),
                        in0=t2f[:, :, :, 0:1], in1=t2f[:, :, :, 1:2])
                    ewс = ew4_c[:, c0 : c0 + ncw].to_broadcast([P, ncw, 4])
                    biasp = smallp.tile([P, ncw * 4], F32, tag="biasp")
                    nc.vector.tensor_mul(
                        out=biasp[:].rearrange("p (c h) -> p c h", h=4),
                        in0=ewс,
                        in1=we_c[:, 0 : ncw * 4].rearrange("p (c h) -> p c h", h=4))
                    if has_be:
                        nc.vector.tensor_add(out=biasp[:], in0=biasp[:],
                                             in1=be_c[:, 0 : ncw * 4])
                    ebx = smallp.tile([P, ncw * 4], F32, tag="ebx")
                    nc.vector.tensor_scalar_mul(out=ebx[:], in0=biasp[:],
                                                scalar1=NEG_SLOPE)
                    nc.vector.tensor_tensor(out=ebx[:], in0=biasp[:], in1=ebx[:],
                                            op=OP.max)
                    nc.scalar.activation(out=ebx[:], in_=ebx[:], func=AF.Exp)
                    el = smallp.tile([P, ncw * 4], F32, tag="el")
                    nc.scalar.activation(
                        out=el[:],
                        in_=logits[:].rearrange("p c h -> p (c h)"),
                        func=AF.Exp, scale=INV_S,
                    )
                    attn_e = smallp.tile([P, ncw * 4], F32, tag="attne")
                    nc.vector.tensor_mul(out=attn_e[:], in0=el[:], in1=ebx[:])
                    attn_w = smallp.tile([P, ncw * 4], F32, tag="attnw")
                    nc.vector.tensor_mul(
                        out=attn_w[:].rearrange("p (c h) -> p c h", h=4),
                        in0=attn_e[:].rearrange("p (c h) -> p c h", h=4), in1=ewс)
                    ae_b = smallp.tile([P, ncw * 4], BF, tag="aeb")
                    nc.vector.tensor_copy(out=ae_b[:], in_=attn_e[:])
                    aw_x = workp.tile([P, ncw, D], BF, tag="awx")
                    nc.scalar.copy(
                        out=aw_x[:].rearrange("p c (h f) -> p c h f", f=D // 4),
                        in_=attn_w[:]
                        .rearrange("p (c h) -> p c h", h=4)
                        .to_broadcast([P, ncw, 4, D // 4]),
                    )
                    rhs = workp.tile([P, ncw, D + 4], BF, tag="rhs")
                    nc.vector.tensor_mul(out=rhs[:, :, 0:D], in0=aw_x[:],
                                         in1=qv_g[:, :, D : 2 * D])
                    nc.vector.tensor_copy(
                        out=rhs[:, :, D : D + 4],
                        in_=ae_b[:].rearrange("p (c h) -> p c h", h=4),
                    )

                    acc = accps.tile([P, D + 4], F32, tag="acc")
                    for c in range(ncw):
                        oh = ohp.tile([P, P], BF, tag="oh")
                        nc.vector.tensor_scalar(
                            out=oh[:], in0=iota_c[:],
                            scalar1=tgtrel_f[:, c0 + c : c0 + c + 1], scalar2=None,
                            op0=OP.is_equal,
                        )
                        nc.tensor.matmul(out=acc[:], lhsT=oh[:], rhs=rhs[:, c, :],
                                         start=(c == 0), stop=(c == ncw - 1))

                    r4 = smallp.tile([P, 4], F32, tag="r4")
                    nc.vector.tensor_scalar_add(out=r4[:], in0=acc[:, D : D + 4],
                                                scalar1=1e-16)
                    nc.vector.reciprocal(out=r4[:], in_=r4[:])
                    mn = smallp.tile([P, D], BF, tag="mn")
                    nc.vector.tensor_tensor(
                        out=mn[:].rearrange("p (h f) -> p h f", h=4),
                        in0=acc[:, 0:D].rearrange("p (h f) -> p h f", h=4),
                        in1=r4[:].to_broadcast([P, 4, D // 4]),
                        op=OP.mult,
                    )
                    pst = tailps.tile([P, P], BF, tag="pst")
                    nc.tensor.transpose(out=pst[:], in_=mn[:], identity=ident[:])
                    mT = smallp.tile([P, P], BF, tag="mT")
                    nc.scalar.copy(out=mT[:], in_=pst[:])
                    pso = tailps.tile([P, D], F32, tag="pso")
                    nc.tensor.matmul(out=pso[:], lhsT=mT[:], rhs=wo_t[:],
                                     start=True, stop=True)
                    res = smallp.tile([P, D], F32, tag="res")
                    res2 = smallp.tile([P, D], F32, tag="res2")
                    if has_bo:
                        nc.vector.tensor_add(out=res[:], in0=pso[:], in1=bo_c[:])
                        nc.scalar.mul(out=res2[:], in_=res[:], mul=NEG_SLOPE)
                        nc.vector.tensor_tensor(out=res[:], in0=res[:], in1=res2[:],
                                                op=OP.max)
                    else:
                        nc.scalar.mul(out=res2[:], in_=pso[:], mul=NEG_SLOPE)
                        nc.vector.tensor_tensor(out=res[:], in0=pso[:], in1=res2[:],
                                                op=OP.max)
                    nc.sync.dma_start(out=out[w * P : (w + 1) * P, :], in_=res[:])

                    c0 += ncw
                    i16 += ncw * 8

    nc.compile()
    return nc


def kernel(h, edge_index, edge_weight, Wq, bq, Wk, bk, Wv, bv, Wo, bo, We, be,
           _run=None):
    from concourse.bass_utils import run_bass_kernel_spmd

    h = np.asarray(h, dtype=np.float32)
    Wq, Wk, Wv, Wo = (np.asarray(x, dtype=np.float32) for x in (Wq, Wk, Wv, Wo))
    bq, bk, bv, bo = (np.asarray(x, dtype=np.float32) for x in (bq, bk, bv, bo))
    We = np.asarray(We, dtype=np.float32).reshape(1, 4)
    be = np.asarray(be, dtype=np.float32)
    V, D = h.shape

    cores, meta = _prep(h, edge_index, edge_weight)
    VPC, NW, VTPAD = meta["VPC"], meta["NW"], meta["VTPAD"]
    KROWS = NW * P
    nchmax = meta["nchmax"]

    has_bqkv = any(np.any(b != 0) for b in (bq, bk, bv))
    has_bo = bool(np.any(bo != 0))
    nc = _build(meta, has_bqkv, has_bo, has_be=bool(np.any(be != 0)))

    iota_np = np.tile(np.arange(P, dtype=np.float32)[None, :], (P, 1)).astype(
        mybir.dt.np(BF)
    )
    we_np = np.tile(We.reshape(1, 4), (P, nchmax)).astype(np.float32)
    be_np = np.tile(be.reshape(1, 4), (P, nchmax)).astype(np.float32)
    bqkv_np = np.tile(
        np.concatenate([bq, bk, bv]).reshape(1, 3 * D), (P, 1)
    ).astype(np.float32)
    bo_np = np.tile(bo.reshape(1, D), (P, 1)).astype(np.float32)
    common = dict(
        Wqv=np.concatenate([Wq, Wv], axis=1).astype(mybir.dt.np(BF)),
        Wk=Wk.astype(mybir.dt.np(BF)),
        Wo=Wo.astype(mybir.dt.np(BF)),
        We_t=we_np, be_t=be_np, bqkv_t=bqkv_np, bo_t=bo_np, iota_b=iota_np,
    )
    in_maps = []
    hbf = h.T.astype(mybir.dt.np(BF))
    for c in range(NCORES):
        hrot = np.zeros((P, VTPAD), dtype=mybir.dt.np(BF))
        r = c * VPC
        hrot[:, : V - r] = hbf[:, r:]
        hrot[:, V - r : V] = hbf[:, :r]
        m = dict(common)
        m.update(
            hT=hrot,
            src16=cores[c]["src16"],
            k16=cores[c]["k16"],
            tgtrel=cores[c]["tgtrel"],
            ew4=cores[c]["ew4"],
        )
        in_maps.append(m)

    if _run is not None:          # test hook (CoreSim etc.)
        return _run(nc, in_maps, meta)

    # the axon transport occasionally fails transiently; one retry is cheap
    try:
        res = run_bass_kernel_spmd(nc, in_maps, core_ids=list(range(NCORES)))
    except Exception:
        res = run_bass_kernel_spmd(nc, in_maps, core_ids=list(range(NCORES)))
    outs = [r["out"][:VPC] for r in res.results]
    return np.concatenate(outs, axis=0)[:V].astype(np.float32)



# revision 40
# speedup vs baseline: 1.0105x; 1.0030x over previous
"""GAT layer on 8 trn2 NeuronCores (Bass/Tile).

Sharding: edges sorted by target node; each core owns a contiguous range of
V/8 target nodes and every edge pointing into it, so attention normalizers
and message sums are core-local (no all-reduce). Node features are projected
into per-core Q/V tables (full, replicated compute) and a core-local K table.

Per core:
  phase 1: TensorE computes Q|V rows (bf16, interleaved 512B records) for all
           nodes into a DRAM table, plus K rows for the core's own range.
  phase 2: per 128-target-node window: dma_gather QV[src] (table split at row
           32768 for the int16 gather index; window edges are grouped into
           low/high chunks of 128) and K[tgt] (rebased core-local indices);
           VectorE: per-edge logit = sum(Q[src]*K[tgt]) per head, bias =
           leaky_relu(ew*We+be), attention exp, message scaling; one-hot
           matrices (is_equal against an iota row) drive TensorE scatter
           matmuls accumulating [message | attn_exp] into PSUM per window;
           then normalize by degree, W_o matmul, leaky_relu, DMA out.

Host does only data movement: sorting/sharding/padding, dtype casts,
index wrapping, output concatenation.
"""

import sys, types, math
import numpy as np

try:
    import antenv.axon_hooks  # noqa: F401
except Exception:
    import antenv  # noqa: F401
    _ah = types.ModuleType("antenv.axon_hooks")
    _ah.get_axon_ntff_profile_hook = lambda: None
    sys.modules["antenv.axon_hooks"] = _ah

import concourse.bass as bass
import concourse.mybir as mybir
import concourse.tile as tile
from concourse import bacc
from concourse.masks import make_identity

P = 128
NCORES = 8
SPLIT = 32768
NEG_SLOPE = 0.2
BF = mybir.dt.bfloat16
F32 = mybir.dt.float32
I16 = mybir.dt.int16
AX = mybir.AxisListType
AF = mybir.ActivationFunctionType
OP = mybir.AluOpType
ABLATE = set()  # {'gather','scatter','dve','phase1'} for timing experiments


def _wrap_idx(pos_idx):
    """dma_gather idx layout: position i -> (partition i%16, col i//16),
    replicated across the 8 Q7 cores (128 partitions)."""
    n = len(pos_idx)
    n16 = (n + 15) // 16
    flat = np.zeros(n16 * 16, dtype=np.int16)
    flat[:n] = pos_idx
    w = flat.reshape(n16, 16).T.copy()
    return np.tile(w, (8, 1))


def _prep(h, edge_index, edge_weight):
    V, D = h.shape
    src = np.asarray(edge_index[0], dtype=np.int64)
    tgt = np.asarray(edge_index[1], dtype=np.int64)
    ew = np.asarray(edge_weight, dtype=np.float32)

    VPC = (V + NCORES - 1) // NCORES
    NW = (VPC + P - 1) // P
    KROWS = ((VPC + P - 1) // P) * P
    VTPAD = ((V + 1023) // 1024) * 1024

    order = np.argsort(tgt, kind="stable")
    s_src, s_tgt, s_ew = src[order], tgt[order], ew[order]
    core_id = s_tgt // VPC
    win_id = (s_tgt % VPC) // P

    lists = [[[None, None] for _ in range(NW)] for _ in range(NCORES)]
    for c in range(NCORES):
        m_c = core_id == c
        cs, ct, cw_, cwin = s_src[m_c], s_tgt[m_c], s_ew[m_c], win_id[m_c]
        # per-core rotated table position: own node range sits at col 0, so
        # the kernel reads K-projection inputs from hT at fixed offsets
        cs = (cs - c * VPC) % V
        for w in range(NW):
            m_w = cwin == w
            ws, wt, we_ = cs[m_w], ct[m_w], cw_[m_w]
            lo = ws < SPLIT
            lists[c][w][0] = (ws[lo], wt[lo], we_[lo])
            lists[c][w][1] = (ws[~lo] - SPLIT, wt[~lo], we_[~lo])

    nch = np.zeros((NW, 2), dtype=np.int64)
    nexact = np.zeros((NW, 2), dtype=np.int64)
    for w in range(NW):
        for hlf in range(2):
            mx = max(len(lists[c][w][hlf][0]) for c in range(NCORES))
            nch[w, hlf] = max(1, (mx + P - 1) // P)
            # exact gather count (max over cores, 16-aligned for the idx wrap)
            nexact[w, hlf] = max(16, ((mx + 15) // 16) * 16)
    nch_tot = int(nch.sum())
    nchmax = int(nch.sum(axis=1).max())
    # process big windows first: short dependency chains drain the pipeline
    worder = np.argsort(-(nch[:, 0] + nch[:, 1]), kind="stable")

    cores = []
    for c in range(NCORES):
        src16_cols, k16_cols = [], []
        tgtrel = np.full((P, nch_tot), -1000.0, dtype=np.float32)
        ew4 = np.zeros((P, nch_tot), dtype=np.float32)
        ccol = 0
        for w in worder:
            kidx_all = []
            for hlf in range(2):
                ws, wt, we_ = lists[c][w][hlf]
                n_slots = int(nch[w, hlf]) * P
                sl_src = np.zeros(n_slots, dtype=np.int64)
                sl_src[: len(ws)] = ws
                sl_rel = np.full(n_slots, -1e9, dtype=np.float32)
                sl_rel[: len(wt)] = (wt - (c * VPC + w * P)).astype(np.float32)
                sl_ew = np.zeros(n_slots, dtype=np.float32)
                sl_ew[: len(we_)] = we_
                sl_k = np.zeros(n_slots, dtype=np.int64)
                sl_k[: len(wt)] = wt - c * VPC
                # match the (g p t) row order of the on-device Ktab writes
                kv = sl_k[: len(wt)]
                main = kv < (KROWS // 512) * 512
                kv_m = kv[main]
                kv[main] = (kv_m // 512) * 512 + (kv_m % 128) * 4 + (kv_m // 128) % 4
                sl_k[: len(wt)] = kv
                src16_cols.append(_wrap_idx(sl_src))
                kidx_all.append(sl_k)
                for j in range(int(nch[w, hlf])):
                    tgtrel[:, ccol + j] = sl_rel[j * P : (j + 1) * P]
                    ew4[:, ccol + j] = sl_ew[j * P : (j + 1) * P]
                ccol += int(nch[w, hlf])
            k16_cols.append(_wrap_idx(np.concatenate(kidx_all)))
        cores.append(
            dict(
                src16=np.ascontiguousarray(np.concatenate(src16_cols, axis=1)),
                k16=np.ascontiguousarray(np.concatenate(k16_cols, axis=1)),
                tgtrel=tgtrel,
                ew4=ew4,
            )
        )

    meta = dict(
        V=V, D=D, VPC=VPC, NW=NW, VTPAD=VTPAD, nch=nch, nch_tot=nch_tot,
        nchmax=nchmax, idx_cols=nch_tot * 8, nexact=nexact, worder=worder,
    )
    return cores, meta


def _build(meta, has_bqkv, has_bo, has_be=True):
    V, D = meta["V"], meta["D"]
    VPC, NW, VTPAD = meta["VPC"], meta["NW"], meta["VTPAD"]
    nch, nchmax, nch_tot = meta["nch"], meta["nchmax"], meta["nch_tot"]
    KROWS = NW * P
    INV_S = 1.0 / math.sqrt(D // 4)
    IDXC = meta["idx_cols"]

    nc = bacc.Bacc(None, target_bir_lowering=False)

    hT = nc.declare_dram_parameter("hT", [P, VTPAD], BF, isOutput=False)
    Wqv = nc.declare_dram_parameter("Wqv", [P, 2 * D], BF, isOutput=False)
    Wk = nc.declare_dram_parameter("Wk", [P, D], BF, isOutput=False)
    Wo = nc.declare_dram_parameter("Wo", [P, D], BF, isOutput=False)
    We_t = nc.declare_dram_parameter("We_t", [P, nchmax * 4], F32, isOutput=False)
    be_t = nc.declare_dram_parameter("be_t", [P, nchmax * 4], F32, isOutput=False)
    bqkv_t = nc.declare_dram_parameter("bqkv_t", [P, 3 * D], F32, isOutput=False)
    bo_t = nc.declare_dram_parameter("bo_t", [P, D], F32, isOutput=False)
    iota_b = nc.declare_dram_parameter("iota_b", [P, P], BF, isOutput=False)
    src16 = nc.declare_dram_parameter("src16", [P, IDXC], I16, isOutput=False)
    k16 = nc.declare_dram_parameter("k16", [P, IDXC], I16, isOutput=False)
    tgtrel = nc.declare_dram_parameter("tgtrel", [P, nch_tot], F32, isOutput=False)
    ew4 = nc.declare_dram_parameter("ew4", [P, nch_tot], F32, isOutput=False)
    out = nc.declare_dram_parameter("out", [KROWS, D], F32, isOutput=True)

    QVtab = nc.dram_tensor("QVtab", [VTPAD, 2 * D], BF)
    Ktab = nc.dram_tensor("Ktab", [KROWS, D], BF)
    NT = VTPAD // P
    KT = KROWS // P

    with tile.TileContext(nc) as tc:
        with tc.tile_pool(name="const", bufs=1) as constp:
            wqv_t = constp.tile([P, 2 * D], BF)
            wk_t = constp.tile([P, D], BF)
            wo_t = constp.tile([P, D], BF)
            we_c = constp.tile([P, nchmax * 4], F32)
            be_c = constp.tile([P, nchmax * 4], F32)
            iota_c = constp.tile([P, P], BF)
            ident = constp.tile([P, P], BF)
            src16_c = constp.tile([P, IDXC], I16)
            k16_c = constp.tile([P, IDXC], I16)
            tgtrel_f = constp.tile([P, nch_tot], F32)
            ew4_c = constp.tile([P, nch_tot], F32)
            nc.sync.dma_start(out=wqv_t[:], in_=Wqv[:])
            make_identity(nc, ident[:])
            bqkv_c = constp.tile([P, 3 * D], F32)
            bo_c = constp.tile([P, D], F32)

            # ---------------- phase 1: projection tables ----------------
            with (
                tc.tile_pool(name="hbuf", bufs=1) as hbufp,
                tc.tile_pool(name="p1ps", bufs=3, space="PSUM") as p1ps,
                tc.tile_pool(name="p1psk", bufs=1, space="PSUM") as p1psk,
                tc.tile_pool(name="p1sb", bufs=10) as p1sb,
            ):
                hT_s = hbufp.tile([P, VTPAD], BF)
                NSLAB = 32
                slab = VTPAD // NSLAB
                for s in range(NSLAB):
                    nc.sync.dma_start(out=hT_s[:, s * slab : (s + 1) * slab],
                                      in_=hT[:, s * slab : (s + 1) * slab])
                # phase-2 constants: loaded after the h slabs so they never
                # delay the phase-1 critical path
                nc.sync.dma_start(out=wk_t[:], in_=Wk[:])
                nc.sync.dma_start(out=wo_t[:], in_=Wo[:])
                nc.sync.dma_start(out=we_c[:], in_=We_t[:])
                nc.sync.dma_start(out=be_c[:], in_=be_t[:])
                nc.sync.dma_start(out=iota_c[:], in_=iota_b[:])
                nc.sync.dma_start(out=src16_c[:], in_=src16[:])
                nc.sync.dma_start(out=k16_c[:], in_=k16[:])
                nc.sync.dma_start(out=tgtrel_f[:], in_=tgtrel[:])
                nc.sync.dma_start(out=ew4_c[:], in_=ew4[:])
                if has_bqkv:
                    nc.sync.dma_start(out=bqkv_c[:], in_=bqkv_t[:])
                if has_bo:
                    nc.sync.dma_start(out=bo_c[:], in_=bo_t[:])
                QVtab_r = QVtab.rearrange("(g t p) d -> g p t d", t=4, p=P)
                for g in range(NT // 4):
                    stage = p1sb.tile([P, 4, 2 * D], BF, tag="qvsb")
                    ps = p1ps.tile([P, 4, 2 * D], F32, tag="qvps")
                    for tt in range(4):
                        t = g * 4 + tt
                        lhs = hT_s[:, t * P : (t + 1) * P]
                        nc.tensor.matmul(out=ps[:, tt, :], lhsT=lhs, rhs=wqv_t[:],
                                         start=True, stop=True)
                    if has_bqkv:
                        for tt in range(4):
                            nc.vector.tensor_add(out=ps[:, tt, 0:D],
                                                 in0=ps[:, tt, 0:D],
                                                 in1=bqkv_c[:, 0:D])
                            nc.vector.tensor_add(out=ps[:, tt, D : 2 * D],
                                                 in0=ps[:, tt, D : 2 * D],
                                                 in1=bqkv_c[:, 2 * D : 3 * D])
                    if g % 2 == 0:
                        nc.scalar.copy(out=stage[:], in_=ps[:])
                    else:
                        nc.vector.tensor_copy(out=stage[:], in_=ps[:])
                    nc.sync.dma_start(out=QVtab_r[g], in_=stage[:])
                KG = KT // 4
                # (g p t) row order: each partition writes 4 consecutive
                # 256B rows = 1024B descriptors (no sub-512B DMA penalty)
                Ktab_r = (
                    Ktab[0 : KG * 4 * P, :].rearrange("(g p t) d -> g p t d", t=4, p=P)
                    if KG > 0 else None
                )
                for g in range(KG):
                    stage = p1sb.tile([P, 4, D], BF, tag="ksb")
                    ps = p1psk.tile([P, 4, D], F32, tag="kps")
                    for tt in range(4):
                        t = g * 4 + tt
                        lhs = hT_s[:, t * P : (t + 1) * P]
                        nc.tensor.matmul(out=ps[:, tt, :], lhsT=lhs, rhs=wk_t[:],
                                         start=True, stop=True)
                    if has_bqkv:
                        for tt in range(4):
                            nc.vector.tensor_add(out=ps[:, tt, :], in0=ps[:, tt, :],
                                                 in1=bqkv_c[:, D : 2 * D])
                    if g % 2 == 0:
                        nc.scalar.copy(out=stage[:], in_=ps[:])
                    else:
                        nc.vector.tensor_copy(out=stage[:], in_=ps[:])
                    nc.sync.dma_start(out=Ktab_r[g], in_=stage[:])
                for t in range(KG * 4, KT):
                    ps = p1psk.tile([P, D], F32, tag="kps")
                    lhs = hT_s[:, t * P : (t + 1) * P]
                    nc.tensor.matmul(out=ps[:], lhsT=lhs, rhs=wk_t[:],
                                     start=True, stop=True)
                    if has_bqkv:
                        nc.vector.tensor_add(out=ps[:], in0=ps[:],
                                             in1=bqkv_c[:, D : 2 * D])
                    kv = p1sb.tile([P, D], BF, tag="ksb1")
                    nc.vector.tensor_copy(out=kv[:], in_=ps[:])
                    nc.sync.dma_start(out=Ktab[t * P : (t + 1) * P, :], in_=kv[:])

            # ---------------- phase 2: edge processing ----------------
            nexact = meta["nexact"]
            with (
                tc.tile_pool(name="gth", bufs=5) as gthp,
                tc.tile_pool(name="work", bufs=3) as workp,
                tc.tile_pool(name="small", bufs=6) as smallp,
                tc.tile_pool(name="oh", bufs=8) as ohp,
                tc.tile_pool(name="accps", bufs=3, space="PSUM") as accps,
                tc.tile_pool(name="tailps", bufs=2, space="PSUM") as tailps,
            ):
                # tables must land in DRAM before any gather reads them
                tc.strict_bb_all_engine_barrier()

                qv_lo = QVtab[0:SPLIT, :]
                qv_hi = QVtab[SPLIT:VTPAD, :]
                c0 = 0      # chunk column cursor
                i16 = 0     # idx column cursor (same for src16 / k16)
                for w in meta["worder"]:
                    nl, nh = int(nch[w, 0]), int(nch[w, 1])
                    nel, neh = int(nexact[w, 0]), int(nexact[w, 1])
                    ncw = nl + nh
                    qv_g = gthp.tile([P, ncw, 2 * D], BF, tag="qvg")
                    k_g = gthp.tile([P, ncw, D], BF, tag="kg")
                    nc.gpsimd.dma_gather(
                        out_ap=qv_g[:, 0:nl, :], in_ap=qv_lo,
                        idxs_ap=src16_c[:, i16 : i16 + nl * 8],
                        num_idxs=nl * P, num_idxs_reg=nl * P, elem_size=2 * D,
                        single_packet=False,
                    )
                    nc.gpsimd.dma_gather(
                        out_ap=qv_g[:, nl:ncw, :], in_ap=qv_hi,
                        idxs_ap=src16_c[:, i16 + nl * 8 : i16 + ncw * 8],
                        num_idxs=nh * P, num_idxs_reg=nh * P, elem_size=2 * D,
                        single_packet=False,
                    )
                    nc.gpsimd.dma_gather(
                        out_ap=k_g[:], in_ap=Ktab[:],
                        idxs_ap=k16_c[:, i16 : i16 + ncw * 8],
                        num_idxs=ncw * P, num_idxs_reg=ncw * P, elem_size=D,
                        single_packet=False,
                    )

                    qk = workp.tile([P, ncw, D], BF, tag="qk")
                    nc.vector.tensor_mul(out=qk[:], in0=qv_g[:, :, 0:D], in1=k_g[:])
                    qk4 = qk[:].rearrange("p c (h f) -> p c h f", f=D // 4)
                    t16 = workp.tile([P, ncw, 4, 16], BF, tag="t16")
                    nc.vector.tensor_add(out=t16[:], in0=qk4[:, :, :, 0:16],
                                         in1=qk4[:, :, :, 16:32])
                    t8 = smallp.tile([P, ncw, 4, 8], BF, tag="t8")
                    nc.vector.tensor_add(out=t8[:], in0=t16[:, :, :, 0:8],
                                         in1=t16[:, :, :, 8:16])
                    t4f = smallp.tile([P, ncw, 4, 4], BF, tag="t4f")
                    nc.vector.tensor_add(out=t4f[:], in0=t8[:, :, :, 0:4],
                                         in1=t8[:, :, :, 4:8])
                    t2f = smallp.tile([P, ncw, 4, 2], BF, tag="t2f")
                    nc.vector.tensor_add(out=t2f[:], in0=t4f[:, :, :, 0:2],
                                         in1=t4f[:, :, :, 2:4])
                    logits = smallp.tile([P, ncw, 4], BF, tag="logits")
                    nc.vector.tensor_add(
                        out=logits[:].rearrange("p c (h x) -> p c h x", x=1),
                        in0=t2f[:, :, :, 0:1], in1=t2f[:, :, :, 1:2])
                    ewс = ew4_c[:, c0 : c0 + ncw].to_broadcast([P, ncw, 4])
                    biasp = smallp.tile([P, ncw * 4], F32, tag="biasp")
                    nc.vector.tensor_mul(
                        out=biasp[:].rearrange("p (c h) -> p c h", h=4),
                        in0=ewс,
                        in1=we_c[:, 0 : ncw * 4].rearrange("p (c h) -> p c h", h=4))
                    if has_be:
                        nc.vector.tensor_add(out=biasp[:], in0=biasp[:],
                                             in1=be_c[:, 0 : ncw * 4])
                    ebx = smallp.tile([P, ncw * 4], F32, tag="ebx")
                    nc.vector.tensor_scalar_mul(out=ebx[:], in0=biasp[:],
                                                scalar1=NEG_SLOPE)
                    nc.vector.tensor_tensor(out=ebx[:], in0=biasp[:], in1=ebx[:],
                                            op=OP.max)
                    nc.scalar.activation(out=ebx[:], in_=ebx[:], func=AF.Exp)
                    el = smallp.tile([P, ncw * 4], F32, tag="el")
                    nc.scalar.activation(
                        out=el[:],
                        in_=logits[:].rearrange("p c h -> p (c h)"),
                        func=AF.Exp, scale=INV_S,
                    )
                    attn_e = smallp.tile([P, ncw * 4], F32, tag="attne")
                    nc.vector.tensor_mul(out=attn_e[:], in0=el[:], in1=ebx[:])
                    attn_w = smallp.tile([P, ncw * 4], F32, tag="attnw")
                    nc.vector.tensor_mul(
                        out=attn_w[:].rearrange("p (c h) -> p c h", h=4),
                        in0=attn_e[:].rearrange("p (c h) -> p c h", h=4), in1=ewс)
                    aw_x = workp.tile([P, ncw, D], BF, tag="awx")
                    nc.scalar.copy(
                        out=aw_x[:].rearrange("p c (h f) -> p c h f", f=D // 4),
                        in_=attn_w[:]
                        .rearrange("p (c h) -> p c h", h=4)
                        .to_broadcast([P, ncw, 4, D // 4]),
                    )
                    rhs = workp.tile([P, ncw, D + 4], BF, tag="rhs")
                    nc.vector.tensor_mul(out=rhs[:, :, 0:D], in0=aw_x[:],
                                         in1=qv_g[:, :, D : 2 * D])
                    nc.vector.tensor_copy(
                        out=rhs[:, :, D : D + 4],
                        in_=attn_e[:].rearrange("p (c h) -> p c h", h=4),
                    )

                    acc = accps.tile([P, D + 4], F32, tag="acc")
                    for c in range(ncw):
                        oh = ohp.tile([P, P], BF, tag="oh")
                        nc.vector.tensor_scalar(
                            out=oh[:], in0=iota_c[:],
                            scalar1=tgtrel_f[:, c0 + c : c0 + c + 1], scalar2=None,
                            op0=OP.is_equal,
                        )
                        nc.tensor.matmul(out=acc[:], lhsT=oh[:], rhs=rhs[:, c, :],
                                         start=(c == 0), stop=(c == ncw - 1))

                    r4 = smallp.tile([P, 4], F32, tag="r4")
                    nc.vector.tensor_scalar_add(out=r4[:], in0=acc[:, D : D + 4],
                                                scalar1=1e-16)
                    nc.vector.reciprocal(out=r4[:], in_=r4[:])
                    mn = smallp.tile([P, D], BF, tag="mn")
                    nc.vector.tensor_tensor(
                        out=mn[:].rearrange("p (h f) -> p h f", h=4),
                        in0=acc[:, 0:D].rearrange("p (h f) -> p h f", h=4),
                        in1=r4[:].to_broadcast([P, 4, D // 4]),
                        op=OP.mult,
                    )
                    pst = tailps.tile([P, P], BF, tag="pst")
                    nc.tensor.transpose(out=pst[:], in_=mn[:], identity=ident[:])
                    mT = smallp.tile([P, P], BF, tag="mT")
                    nc.scalar.copy(out=mT[:], in_=pst[:])
                    pso = tailps.tile([P, D], F32, tag="pso")
                    nc.tensor.matmul(out=pso[:], lhsT=mT[:], rhs=wo_t[:],
                                     start=True, stop=True)
                    res = smallp.tile([P, D], F32, tag="res")
                    res2 = smallp.tile([P, D], F32, tag="res2")
                    if has_bo:
                        nc.vector.tensor_add(out=res[:], in0=pso[:], in1=bo_c[:])
                        nc.scalar.mul(out=res2[:], in_=res[:], mul=NEG_SLOPE)
                        nc.vector.tensor_tensor(out=res[:], in0=res[:], in1=res2[:],
                                                op=OP.max)
                    else:
                        nc.scalar.mul(out=res2[:], in_=pso[:], mul=NEG_SLOPE)
                        nc.vector.tensor_tensor(out=res[:], in0=pso[:], in1=res2[:],
                                                op=OP.max)
                    nc.sync.dma_start(out=out[w * P : (w + 1) * P, :], in_=res[:])

                    c0 += ncw
                    i16 += ncw * 8

    nc.compile()
    return nc


def kernel(h, edge_index, edge_weight, Wq, bq, Wk, bk, Wv, bv, Wo, bo, We, be,
           _run=None):
    from concourse.bass_utils import run_bass_kernel_spmd

    h = np.asarray(h, dtype=np.float32)
    Wq, Wk, Wv, Wo = (np.asarray(x, dtype=np.float32) for x in (Wq, Wk, Wv, Wo))
    bq, bk, bv, bo = (np.asarray(x, dtype=np.float32) for x in (bq, bk, bv, bo))
    We = np.asarray(We, dtype=np.float32).reshape(1, 4)
    be = np.asarray(be, dtype=np.float32)
    V, D = h.shape

    cores, meta = _prep(h, edge_index, edge_weight)
    VPC, NW, VTPAD = meta["VPC"], meta["NW"], meta["VTPAD"]
    KROWS = NW * P
    nchmax = meta["nchmax"]

    has_bqkv = any(np.any(b != 0) for b in (bq, bk, bv))
    has_bo = bool(np.any(bo != 0))
    nc = _build(meta, has_bqkv, has_bo, has_be=bool(np.any(be != 0)))

    iota_np = np.tile(np.arange(P, dtype=np.float32)[None, :], (P, 1)).astype(
        mybir.dt.np(BF)
    )
    we_np = np.tile(We.reshape(1, 4), (P, nchmax)).astype(np.float32)
    be_np = np.tile(be.reshape(1, 4), (P, nchmax)).astype(np.float32)
    bqkv_np = np.tile(
        np.concatenate([bq, bk, bv]).reshape(1, 3 * D), (P, 1)
    ).astype(np.float32)
    bo_np = np.tile(bo.reshape(1, D), (P, 1)).astype(np.float32)
    common = dict(
        Wqv=np.concatenate([Wq, Wv], axis=1).astype(mybir.dt.np(BF)),
        Wk=Wk.astype(mybir.dt.np(BF)),
        Wo=Wo.astype(mybir.dt.np(BF)),
        We_t=we_np, be_t=be_np, bqkv_t=bqkv_np, bo_t=bo_np, iota_b=iota_np,
    )
    in_maps = []
    hbf = h.T.astype(mybir.dt.np(BF))
    for c in range(NCORES):
        hrot = np.zeros((P, VTPAD), dtype=mybir.dt.np(BF))
        r = c * VPC
        hrot[:, : V - r] = hbf[:, r:]
        hrot[:, V - r : V] = hbf[:, :r]
        m = dict(common)
        m.update(
            hT=hrot,
            src16=cores[c]["src16"],
            k16=cores[c]["k16"],
            tgtrel=cores[c]["tgtrel"],
            ew4=cores[c]["ew4"],
        )
        in_maps.append(m)

    if _run is not None:          # test hook (CoreSim etc.)
        return _run(nc, in_maps, meta)

    # the axon transport occasionally fails transiently; one retry is cheap
    try:
        res = run_bass_kernel_spmd(nc, in_maps, core_ids=list(range(NCORES)))
    except Exception:
        res = run_bass_kernel_spmd(nc, in_maps, core_ids=list(range(NCORES)))
    outs = [r["out"][:VPC] for r in res.results]
    return np.concatenate(outs, axis=0)[:V].astype(np.float32)

